# revision 6
# baseline (speedup 1.0000x reference)
"""TRN2 Bass/Tile kernel for nn_NoFoDifformer (8-core SPMD, row-sharded).

Per core m (rows R_m = [m*1024, (m+1)*1024)):
  - All inputs arrive in ONE packed buffer (pk) per core: u is pre-cast
    host-side to fp8 e4m3 (x32 prescale) in BOTH layouts (u8 row-major for
    pass 1, uT8 column-major for pass 2), x is pre-transposed. The single
    buffer keeps the per-dispatch operand count at 2 (pk + y).
  - pass 1 (utx^T += (h/32)^T @ u8_panel): fp8 panels stream on the Pool
    SWDGE queue (0.5 MB each); matmuls at bf16 rate; psum copied to bf16
    utxT (exact scale: the 1/32 on h cancels the x32 on u).
  - new_e (sine eigen-encoding, Cody-Waite + ACT Sin) is jt-sharded, emitted
    first (DVE-only start), AllGathered early; the pass-2 1/32 descale is
    folded into its weights.
  - utx^T is all-reduced in bf16 in KERNEL_NAR (default 4) pipelined chunks
    as their panels complete; k^T v and new_e ride small collectives
    staggered into the Pool FIFO between panel loads.
  - pass 2 (h_fur^T += g16^T @ uT8): uT8 is DMA'd once (8.4 MB) into SBUF;
    per chunk as the AllReduce lands: g16 = transpose(utxT tile)*(new_e/32),
    bf16 x fp8 matmuls. No on-chip u transposes at all.
  - fp8 on the u path contributes ~5e-5 of the output scale (gate 2e-2);
    the attention path stays fp32.
"""

import numpy as np

import concourse.bacc as bacc
import concourse.mybir as mybir
import concourse.tile as tile
from concourse.bass_utils import run_bass_kernel_spmd
from concourse.masks import make_identity

F32 = mybir.dt.float32
BF16 = mybir.dt.bfloat16
FP8 = mybir.dt.float8e4
AF = mybir.ActivationFunctionType
ALU = mybir.AluOpType

NCORES = 8
N = 8192
NFEAT = 512
HID = 256
C = 128
DIM = 32
KPOW = 10
ROWS = N // NCORES      # 1024 rows per core
NT = ROWS // 128        # 8 row tiles
JT = N // 128           # 64 column tiles
PW = 512                # pass-1 panel width
JP = N // PW            # 16 panels
JTC = JT // NCORES      # 8 jt per core for new_e sharding
LN_EPS = 1e-5

U8_SCALE = 32.0         # fp8 prescale on u; cancelled via h/32 and new_e/32

TWO_PI = 6.283185307179586
INV_2PI = 1.0 / TWO_PI
CW_C1 = 6.28125
CW_C2 = float(np.float32(TWO_PI - CW_C1))
CW_C3 = TWO_PI - CW_C1 - CW_C2
MAGIC = 12582912.0      # 1.5 * 2**23, round-to-nearest trick
HALF_PI = float(np.float32(np.pi / 2))
PI_F = float(np.float32(np.pi))

WEIGHT_NAMES = [
    ("fe_w1", [NFEAT, HID]), ("fe_b1", [HID]),
    ("fe_w2", [HID, C]), ("fe_b2", [C]),
    ("eig_w", [KPOW, DIM + 1]), ("eig_b", [KPOW]), ("alpha_w", [KPOW]),
    ("mha_g", [C]), ("mha_b", [C]), ("ffn_g", [C]), ("ffn_b", [C]),
    ("wq", [C, C]), ("bq", [C]), ("wk", [C, C]), ("bk", [C]),
    ("wv", [C, C]), ("bv", [C]), ("wo", [C, C]), ("bo", [C]),
    ("f1_w", [C, C]), ("f1_b", [C]), ("f2_w", [C, C]), ("f2_b", [C]),
]

# Packed fp32-typed input layout (offsets in fp32 slots). The two u copies
# are fp8 bytes bitcast-viewed on device; both are 4B-aligned.
U8_SLOTS = ROWS * N // 4
_PACK_FIELDS = [
    ("u8_s", U8_SLOTS),          # fp8(32*u_s), row-major [ROWS, N]
    ("uT8_s", U8_SLOTS),         # fp8(32*u_s.T), [N, ROWS]
    ("xT_s", NFEAT * ROWS),
    ("e_js", 128 * JTC),
] + [(name, int(np.prod(shape))) for name, shape in WEIGHT_NAMES]
_PACK_OFF = {}
_off = 0
for _n, _s in _PACK_FIELDS:
    _PACK_OFF[_n] = _off
    _off += _s
PACK_TOTAL = _off
_PACK_SIZE = dict(_PACK_FIELDS)


def _build(nc):
    pk = nc.dram_tensor("pk", [PACK_TOTAL], F32, kind="ExternalInput")

    def io_ap(name):
        off = _PACK_OFF[name]
        return pk.ap()[off:off + _PACK_SIZE[name]]

    y = nc.dram_tensor("y", [ROWS, C], F32, kind="ExternalOutput")

    div_const = nc.inline_tensor(
        np.tile(np.arange(1, DIM // 2 + 1, dtype=np.float32), (128, 1)), name="divc"
    )

    with tile.TileContext(nc) as tc:
        with (
            tc.tile_pool(name="persist", bufs=1) as per,
            tc.tile_pool(name="pan", bufs=3) as pan,
            tc.tile_pool(name="rot", bufs=3) as rot,
            tc.tile_pool(name="g16p", bufs=4) as g16_pool,
            tc.tile_pool(name="stats", bufs=4) as stats,
            tc.tile_pool(name="psum_sm", bufs=3, space="PSUM") as psum_sm,
            tc.tile_pool(name="psum_tr", bufs=3, space="PSUM") as psum_tr,
            tc.tile_pool(name="psum_acc", bufs=1, space="PSUM") as psum_acc,
            tc.tile_pool(name="dram", bufs=1, space="DRAM") as dram,
        ):
            import os as _os
            _REPL = int(_os.environ.get("KERNEL_REPLICATE", "1"))
            NAR = int(_os.environ.get("KERNEL_NAR", "4"))
            JPC = JP // NAR              # panels per chunk
            CW = N // NAR                # columns per chunk
            JTCW = CW // 128             # jt tiles per chunk

            def _body_once():
                rg = [list(range(NCORES))]

                # ---------------- constants / weights to SBUF ----------------
                ident = per.tile([128, 128], F32, tag="ident")
                make_identity(nc, ident[:])
                ident16 = per.tile([128, 128], BF16, tag="ident16")
                make_identity(nc, ident16[:])

                eps_sb = per.tile([128, 1], F32, tag="eps_sb")
                nc.vector.memset(eps_sb[:], LN_EPS)

                div_sb = per.tile([128, DIM // 2], F32, tag="div_sb")
                nc.scalar.dma_start(out=div_sb[:], in_=div_const.ap())

                def bcast(name, width, tag):
                    t = per.tile([128, width], F32, tag=tag)
                    nc.scalar.dma_start(out=t[:], in_=io_ap(name).partition_broadcast(128))
                    return t

                def per_part(name, tag):
                    t = per.tile([128, 1], F32, tag=tag)
                    nc.scalar.dma_start(out=t[:], in_=io_ap(name).rearrange("(p o) -> p o", o=1))
                    return t

                # ---------------- new_e (jt-sharded) first: DVE-only start ----------------
                eigw_bc = bcast("eig_w", KPOW * (DIM + 1), "eigw_bc")
                eigb_bc = bcast("eig_b", KPOW, "eigb_bc")
                alpha_bc = bcast("alpha_w", KPOW, "alpha_bc")

                w2s = per.tile([128, KPOW, DIM // 2], F32, tag="w2s")
                w2c = per.tile([128, KPOW, DIM // 2], F32, tag="w2c")
                eigw_3d = eigw_bc[:].rearrange("p (k d) -> p k d", d=DIM + 1)
                alpha_b3 = alpha_bc[:].unsqueeze(2).broadcast_to([128, KPOW, DIM // 2])
                nc.vector.tensor_tensor(out=w2s[:], in0=alpha_b3, in1=eigw_3d[:, :, 1:1 + DIM // 2], op=ALU.mult)
                nc.vector.tensor_tensor(out=w2c[:], in0=alpha_b3, in1=eigw_3d[:, :, 1 + DIM // 2:DIM + 1], op=ALU.mult)
                # fold the fp8 u descale (1/U8_SCALE) into the sine weights
                nc.vector.tensor_scalar_mul(out=w2s[:], in0=w2s[:], scalar1=1.0 / U8_SCALE)
                nc.vector.tensor_scalar_mul(out=w2c[:], in0=w2c[:], scalar1=1.0 / U8_SCALE)
                w0t = per.tile([128, KPOW], F32, tag="w0t")
                nc.vector.tensor_tensor(out=w0t[:], in0=eigw_3d[:, :, 0], in1=eigb_bc[:], op=ALU.add)
                nc.vector.tensor_tensor(out=w0t[:], in0=w0t[:], in1=alpha_bc[:], op=ALU.mult)
                w0 = per.tile([128, 1], F32, tag="w0")
                nc.vector.tensor_reduce(out=w0[:], in_=w0t[:], axis=mybir.AxisListType.X, op=ALU.add)
                nc.vector.tensor_scalar_mul(out=w0[:], in0=w0[:], scalar1=1.0 / U8_SCALE)

                e_sb = per.tile([128, JTC], F32, tag="e_sb")
                nc.scalar.dma_start(out=e_sb[:], in_=io_ap("e_js").rearrange("(p w) -> p w", p=128))
                pows = per.tile([128, JTC, KPOW], F32, tag="pows")
                nc.vector.tensor_copy(out=pows[:, :, 0], in_=e_sb[:])
                for k in range(1, KPOW):
                    nc.vector.tensor_tensor(out=pows[:, :, k], in0=pows[:, :, k - 1], in1=e_sb[:], op=ALU.mult)

                WNE = JTC * KPOW * (DIM // 2)  # 1280
                pe_t = per.tile([128, JTC, KPOW, DIM // 2], F32, tag="pe_t")
                kq_t = per.tile([128, WNE], F32, tag="kq_t")
                trig = per.tile([128, WNE], F32, tag="trig")
                ne_s = per.tile([128, JTC], F32, tag="ne_s")
                ne_c = per.tile([128, JTC], F32, tag="ne_c")

                pows_b = pows[:].unsqueeze(3).broadcast_to([128, JTC, KPOW, DIM // 2])
                div_b = div_sb[:].unsqueeze(1).unsqueeze(1).broadcast_to([128, JTC, KPOW, DIM // 2])
                nc.vector.tensor_tensor(out=pe_t[:], in0=pows_b, in1=div_b, op=ALU.mult)
                pe_f = pe_t[:].rearrange("p a b c -> p (a b c)")
                nc.vector.tensor_scalar(out=kq_t[:], in0=pe_f, scalar1=INV_2PI, scalar2=MAGIC, op0=ALU.mult, op1=ALU.add)
                nc.vector.tensor_scalar_sub(out=kq_t[:], in0=kq_t[:], scalar1=MAGIC)
                # range-reduce pe in place: pe -= k*(c1+c2+c3)
                nc.vector.cody_waite_cascade(pe_f, pe_f, kq_t[:], CW_C1, CW_C2, CW_C3)

                w2s_b = w2s[:].rearrange("p k d -> p (k d)").unsqueeze(1).broadcast_to([128, JTC, KPOW * DIM // 2])
                w2c_b = w2c[:].rearrange("p k d -> p (k d)").unsqueeze(1).broadcast_to([128, JTC, KPOW * DIM // 2])

                nc.scalar.activation(out=trig[:], in_=pe_f, func=AF.Sin)
                trig3 = trig[:].rearrange("p (a w) -> p a w", a=JTC)
                nc.vector.tensor_tensor(out=trig3, in0=trig3, in1=w2s_b, op=ALU.mult)
                nc.vector.tensor_reduce(out=ne_s[:], in_=trig3, axis=mybir.AxisListType.X, op=ALU.add)

                nc.vector.add_range_wrap(kq_t[:], pe_f, HALF_PI, PI_F, TWO_PI)
                nc.scalar.activation(out=trig[:], in_=kq_t[:], func=AF.Sin)
                nc.vector.tensor_tensor(out=trig3, in0=trig3, in1=w2c_b, op=ALU.mult)
                nc.vector.tensor_reduce(out=ne_c[:], in_=trig3, axis=mybir.AxisListType.X, op=ALU.add)

                nc.vector.tensor_tensor(out=ne_s[:], in0=ne_s[:], in1=ne_c[:], op=ALU.add)
                nc.vector.tensor_scalar_add(out=ne_s[:], in0=ne_s[:], scalar1=w0[:])

                ag_in = dram.tile([128 * JTC], F32, tag="ag_in")
                ag_out = dram.tile([N], F32, tag="ag_out", addr_space="Shared")
                new_e_sb = per.tile([128, JT], F32, tag="new_e_sb")

                def emit_ag():
                    nc.sync.dma_start(out=ag_in[:].rearrange("(p w) -> p w", p=128), in_=ne_s[:])
                    nc.gpsimd.collective_compute(
                        "AllGather", ALU.bypass, replica_groups=rg,
                        ins=[ag_in[:].opt()], outs=[ag_out[:].opt()],
                    )
                    nc.scalar.dma_start(
                        out=new_e_sb[:].rearrange("p (m w) -> p m w", w=JTC),
                        in_=ag_out[:].rearrange("(m p w) -> p m w", p=128, w=JTC),
                    )

                # ---------------- encoder: h = relu(x@w1+b1)@w2+b2 ----------------
                w1_sb = per.tile([128, NFEAT // 128, HID], F32, tag="w1_sb")
                nc.sync.dma_start(out=w1_sb[:], in_=io_ap("fe_w1").rearrange("(t p h) -> p t h", p=128, h=HID))
                w2_sb = per.tile([128, HID // 128, C], F32, tag="w2_sb")
                nc.sync.dma_start(out=w2_sb[:], in_=io_ap("fe_w2").rearrange("(t p c) -> p t c", p=128, c=C))
                b1_sb = per.tile([128, HID // 128], F32, tag="b1_sb")
                nc.sync.dma_start(out=b1_sb[:], in_=io_ap("fe_b1").rearrange("(t p) -> p t", p=128))
                b2_bc = bcast("fe_b2", C, "b2_bc")

                wq_sb = per.tile([128, C], F32, tag="wq_sb")
                nc.sync.dma_start(out=wq_sb[:], in_=io_ap("wq").rearrange("(p c) -> p c", c=C))
                wk_sb = per.tile([128, C], F32, tag="wk_sb")
                nc.sync.dma_start(out=wk_sb[:], in_=io_ap("wk").rearrange("(p c) -> p c", c=C))
                wv_sb = per.tile([128, C], F32, tag="wv_sb")
                nc.sync.dma_start(out=wv_sb[:], in_=io_ap("wv").rearrange("(p c) -> p c", c=C))
                wo_sb = per.tile([128, C], F32, tag="wo_sb")
                nc.sync.dma_start(out=wo_sb[:], in_=io_ap("wo").rearrange("(p c) -> p c", c=C))
                f1w_sb = per.tile([128, C], F32, tag="f1w_sb")
                nc.sync.dma_start(out=f1w_sb[:], in_=io_ap("f1_w").rearrange("(p c) -> p c", c=C))
                f2w_sb = per.tile([128, C], F32, tag="f2w_sb")
                nc.sync.dma_start(out=f2w_sb[:], in_=io_ap("f2_w").rearrange("(p c) -> p c", c=C))

                bq_pp = per_part("bq", "bq_pp")
                bo_pp = per_part("bo", "bo_pp")
                f1b_pp = per_part("f1_b", "f1b_pp")
                f2b_pp = per_part("f2_b", "f2b_pp")
                bk_bc = bcast("bk", C, "bk_bc")
                bv_bc = bcast("bv", C, "bv_bc")
                mhag_bc = bcast("mha_g", C, "mhag_bc")
                mhab_bc = bcast("mha_b", C, "mhab_bc")
                ffng_bc = bcast("ffn_g", C, "ffng_bc")
                ffnb_bc = bcast("ffn_b", C, "ffnb_bc")

                # xT arrives pre-transposed: [f_part, 4(ft), n]
                xT = per.tile([128, NFEAT // 128, ROWS], F32, tag="xT")
                nc.sync.dma_start(out=xT[:], in_=io_ap("xT_s").rearrange("(t p n) -> p t n", p=128, n=ROWS))

                # uT8 (pass-2 moving operand) as one 8.4 MB DMA
                uT8 = per.tile([128, JT, ROWS], FP8, tag="uT8")
                nc.sync.dma_start(
                    out=uT8[:],
                    in_=io_ap("uT8_s").bitcast(FP8).rearrange("(t p i) -> p t i", p=128, i=ROWS),
                )

                # t1^T [hid_part, 2(ht), n] = relu(w1^T x^T + b1)
                t1T = per.tile([128, HID // 128, ROWS], F32, tag="t1T")
                for ht in range(HID // 128):
                    for nch in range(ROWS // 512):
                        ps = psum_sm.tile([128, 512], F32, tag="ps_sm")
                        for ft in range(NFEAT // 128):
                            nc.tensor.matmul(
                                ps[:], lhsT=w1_sb[:, ft, ht * 128:(ht + 1) * 128],
                                rhs=xT[:, ft, nch * 512:(nch + 1) * 512],
                                start=(ft == 0), stop=(ft == NFEAT // 128 - 1),
                            )
                        nc.scalar.activation(
                            out=t1T[:, ht, nch * 512:(nch + 1) * 512], in_=ps[:],
                            func=AF.Relu, bias=b1_sb[:, ht:ht + 1],
                        )

                # h [n_part, 8(nt), C] = t1 @ w2 + b2
                h_sb = per.tile([128, NT, C], F32, tag="h_sb")
                for nt in range(NT):
                    ps = psum_sm.tile([128, C], F32, tag="ps_sm")
                    for ht in range(HID // 128):
                        nc.tensor.matmul(
                            ps[:], lhsT=t1T[:, ht, nt * 128:(nt + 1) * 128],
                            rhs=w2_sb[:, ht, :],
                            start=(ht == 0), stop=(ht == HID // 128 - 1),
                        )
                    nc.vector.tensor_add(out=h_sb[:, nt, :], in0=ps[:], in1=b2_bc[:])

                # h16 = h / U8_SCALE: cancels the x32 on the fp8 u panels
                h16_sb = per.tile([128, NT, C], BF16, tag="h16_sb")
                for nt in range(NT):
                    nc.vector.tensor_scalar_mul(out=h16_sb[:, nt, :], in0=h_sb[:, nt, :], scalar1=1.0 / U8_SCALE)

                # ---------------- LN1 + q/k/v + kTv partial ----------------
                def layer_norm(src, dst, g_bc, b_bc):
                    for nt in range(NT):
                        st = stats.tile([128, 6], F32, tag="ln_st")
                        nc.vector.bn_stats(out=st[:], in_=src[:, nt, :])
                        mv = stats.tile([128, 2], F32, tag="ln_mv")
                        nc.vector.bn_aggr(out=mv[:], in_=st[:])
                        rstd = stats.tile([128, 1], F32, tag="ln_rstd")
                        nc.scalar.activation(out=rstd[:], in_=mv[:, 1:2], func=AF.Sqrt, bias=eps_sb[:])
                        nc.vector.reciprocal(out=rstd[:], in_=rstd[:])
                        nc.vector.tensor_scalar(
                            out=dst[:, nt, :], in0=src[:, nt, :],
                            scalar1=mv[:, 0:1], scalar2=rstd[:],
                            op0=ALU.subtract, op1=ALU.mult,
                        )
                        nc.vector.tensor_tensor(out=dst[:, nt, :], in0=dst[:, nt, :], in1=g_bc[:], op=ALU.mult)
                        nc.vector.tensor_tensor(out=dst[:, nt, :], in0=dst[:, nt, :], in1=b_bc[:], op=ALU.add)

                mh_sb = per.tile([128, NT, C], F32, tag="mh_sb")
                layer_norm(h_sb, mh_sb, mhag_bc, mhab_bc)

                mhT = per.tile([128, ROWS], F32, tag="mhT")
                for nt in range(NT):
                    tp = psum_tr.tile([128, 128], F32, tag="tr")
                    nc.tensor.transpose(tp[:], mh_sb[:, nt, :], ident[:])
                    nc.vector.tensor_copy(out=mhT[:, nt * 128:(nt + 1) * 128], in_=tp[:])

                qT = per.tile([128, ROWS], F32, tag="qT")
                for nch in range(ROWS // 512):
                    ps = psum_sm.tile([128, 512], F32, tag="ps_sm")
                    nc.tensor.matmul(ps[:], lhsT=wq_sb[:], rhs=mhT[:, nch * 512:(nch + 1) * 512], start=True, stop=True)
                    nc.scalar.activation(out=qT[:, nch * 512:(nch + 1) * 512], in_=ps[:], func=AF.Identity, bias=bq_pp[:])

                k_sb = per.tile([128, NT, C], F32, tag="k_sb")
                v_sb = per.tile([128, NT, C], F32, tag="v_sb")
                for nt in range(NT):
                    ps = psum_sm.tile([128, C], F32, tag="ps_sm")
                    nc.tensor.matmul(ps[:], lhsT=mhT[:, nt * 128:(nt + 1) * 128], rhs=wk_sb[:], start=True, stop=True)
                    nc.vector.tensor_add(out=k_sb[:, nt, :], in0=ps[:], in1=bk_bc[:])
                    ps2 = psum_sm.tile([128, C], F32, tag="ps_sm")
                    nc.tensor.matmul(ps2[:], lhsT=mhT[:, nt * 128:(nt + 1) * 128], rhs=wv_sb[:], start=True, stop=True)
                    nc.vector.tensor_add(out=v_sb[:, nt, :], in0=ps2[:], in1=bv_bc[:])

                kTv_sb = per.tile([128, C], F32, tag="kTv_sb")
                pskv = psum_sm.tile([128, C], F32, tag="ps_sm")
                for nt in range(NT):
                    nc.tensor.matmul(pskv[:], lhsT=k_sb[:, nt, :], rhs=v_sb[:, nt, :], start=(nt == 0), stop=(nt == NT - 1))
                nc.vector.tensor_copy(out=kTv_sb[:], in_=pskv[:])

                # ---------------- pass 1 + chunked AllReduce ----------------
                utxT = per.tile([128, N], BF16, tag="utxT")
                u8_r = io_ap("u8_s").bitcast(FP8).rearrange("(t p j) -> p t j", p=128, j=N)

                ar_ins, ar_outs = [], []
                for c in range(NAR):
                    ari = dram.tile([128, CW], BF16, tag=f"ar_in{c}", name=f"ar_in{c}")
                    aro = dram.tile([128, CW], BF16, tag=f"ar_out{c}", name=f"ar_out{c}",
                                    addr_space="Shared")
                    ar_ins.append(ari)
                    ar_outs.append(aro)
                ktv_in = dram.tile([128, C], F32, tag="ktv_in")
                ktv_out = dram.tile([128, C], F32, tag="ktv_out", addr_space="Shared")

                def emit_chunk_ar(c):
                    # input copy on sync; trigger on gpsimd (required engine for
                    # collectives); result load-back on scalar HWDGE so the sync
                    # FIFO and Pool FIFO (panel loads) don't stall on completion.
                    nc.sync.dma_start(out=ar_ins[c][:], in_=utxT[:, c * CW:(c + 1) * CW])
                    nc.gpsimd.collective_compute(
                        "AllReduce", ALU.add, replica_groups=rg,
                        ins=[ar_ins[c][:].opt()], outs=[ar_outs[c][:].opt()],
                    )
                    nc.scalar.dma_start(out=utxT[:, c * CW:(c + 1) * CW], in_=ar_outs[c][:])

                def emit_ktv_ar():
                    nc.sync.dma_start(out=ktv_in[:], in_=kTv_sb[:])
                    nc.gpsimd.collective_compute(
                        "AllReduce", ALU.add, replica_groups=rg,
                        ins=[ktv_in[:].opt()], outs=[ktv_out[:].opt()],
                    )
                    nc.scalar.dma_start(out=kTv_sb[:], in_=ktv_out[:])

                for jp in range(JP):
                    panel = pan.tile([128, NT, PW], FP8, tag="panel")
                    nc.gpsimd.dma_start(out=panel[:], in_=u8_r[:, :, jp * PW:(jp + 1) * PW])
                    # stagger collective triggers between panel loads so the
                    # gpsimd sequencer's wait overlaps in-flight panel loads
                    if jp == 2:
                        emit_ag()
                    if jp == 3:
                        emit_ktv_ar()
                    if jp >= JPC + 2 and (jp - JPC - 2) % JPC == 0 and (jp - JPC - 2) // JPC < NAR - 1:
                        emit_chunk_ar((jp - JPC - 2) // JPC)
                    ps = psum_sm.tile([128, PW], F32, tag="ps_sm")
                    for nt in range(NT):
                        nc.tensor.matmul(
                            ps[:], lhsT=h16_sb[:, nt, :],
                            rhs=panel[:, nt, :],
                            start=(nt == 0), stop=(nt == NT - 1),
                        )
                    nc.vector.tensor_copy(out=utxT[:, jp * PW:(jp + 1) * PW], in_=ps[:])
                emit_chunk_ar(NAR - 1)

                # ---------------- pass 2: h_fur^T += g16^T @ uT8 ----------------
                hfur_ps = psum_acc.tile([128, ROWS], F32, tag="hfur")
                for c in range(NAR):
                    for jtl in range(JTCW):
                        jt = c * JTCW + jtl
                        tp = psum_tr.tile([128, 128], BF16, tag="tr", name="tp16")
                        nc.tensor.transpose(tp[:], utxT[:, jt * 128:(jt + 1) * 128], ident16[:])
                        g16 = g16_pool.tile([128, 128], BF16, tag="g16")
                        if jt % 2 == 0:
                            nc.vector.tensor_scalar_mul(out=g16[:], in0=tp[:], scalar1=new_e_sb[:, jt:jt + 1])
                        else:
                            nc.scalar.activation(out=g16[:], in_=tp[:], func=AF.Identity, scale=new_e_sb[:, jt:jt + 1])
                        for hf in range(ROWS // 512):
                            nc.tensor.matmul(
                                hfur_ps[:, hf * 512:(hf + 1) * 512], lhsT=g16[:],
                                rhs=uT8[:, jt, hf * 512:(hf + 1) * 512],
                                start=(jt == 0), stop=(jt == JT - 1),
                                skip_group_check=True,
                            )

                # ---------------- att^T, att2^T + h_fur^T -> s^T; h1 = h + s ----------------
                hfurT = rot.tile([128, ROWS], F32, tag="bigT", name="hfurT")
                nc.vector.tensor_copy(out=hfurT[:], in_=hfur_ps[:])

                attT = rot.tile([128, ROWS], F32, tag="bigT", name="attT")
                for nch in range(ROWS // 512):
                    ps = psum_sm.tile([128, 512], F32, tag="ps_sm")
                    nc.tensor.matmul(ps[:], lhsT=kTv_sb[:], rhs=qT[:, nch * 512:(nch + 1) * 512], start=True, stop=True)
                    nc.vector.tensor_copy(out=attT[:, nch * 512:(nch + 1) * 512], in_=ps[:])

                sT = rot.tile([128, ROWS], F32, tag="bigT", name="sT")
                for nch in range(ROWS // 512):
                    ps = psum_sm.tile([128, 512], F32, tag="ps_sm")
                    nc.tensor.matmul(ps[:], lhsT=wo_sb[:], rhs=attT[:, nch * 512:(nch + 1) * 512], start=True, stop=True)
                    nc.vector.scalar_tensor_tensor(
                        out=sT[:, nch * 512:(nch + 1) * 512], in0=ps[:], scalar=bo_pp[:],
                        in1=hfurT[:, nch * 512:(nch + 1) * 512],
                        op0=ALU.add, op1=ALU.add,
                    )

                h1_sb = per.tile([128, NT, C], F32, tag="h1_sb")
                for nt in range(NT):
                    tp = psum_tr.tile([128, 128], F32, tag="tr")
                    nc.tensor.transpose(tp[:], sT[:, nt * 128:(nt + 1) * 128], ident[:])
                    nc.vector.tensor_add(out=h1_sb[:, nt, :], in0=tp[:], in1=h_sb[:, nt, :])

                # ---------------- FFN: h_out = h1 + (gelu(LN(h1)@f1+b1))@f2+b2 ----------------
                mh2_sb = per.tile([128, NT, C], F32, tag="mh2_sb")
                layer_norm(h1_sb, mh2_sb, ffng_bc, ffnb_bc)
                mh2T = rot.tile([128, ROWS], F32, tag="bigT", name="mh2T")
                for nt in range(NT):
                    tp = psum_tr.tile([128, 128], F32, tag="tr")
                    nc.tensor.transpose(tp[:], mh2_sb[:, nt, :], ident[:])
                    nc.vector.tensor_copy(out=mh2T[:, nt * 128:(nt + 1) * 128], in_=tp[:])

                gzT = rot.tile([128, ROWS], F32, tag="bigT", name="gzT")
                for nch in range(ROWS // 512):
                    ps = psum_sm.tile([128, 512], F32, tag="ps_sm")
                    nc.tensor.matmul(ps[:], lhsT=f1w_sb[:], rhs=mh2T[:, nch * 512:(nch + 1) * 512], start=True, stop=True)
                    nc.scalar.activation(out=gzT[:, nch * 512:(nch + 1) * 512], in_=ps[:], func=AF.Gelu, bias=f1b_pp[:])

                f2T = rot.tile([128, ROWS], F32, tag="bigT", name="f2T")
                for nch in range(ROWS // 512):
                    ps = psum_sm.tile([128, 512], F32, tag="ps_sm")
                    nc.tensor.matmul(ps[:], lhsT=f2w_sb[:], rhs=gzT[:, nch * 512:(nch + 1) * 512], start=True, stop=True)
                    nc.scalar.activation(out=f2T[:, nch * 512:(nch + 1) * 512], in_=ps[:], func=AF.Identity, bias=f2b_pp[:])

                hout_sb = per.tile([128, NT, C], F32, tag="hout_sb")
                for nt in range(NT):
                    tp = psum_tr.tile([128, 128], F32, tag="tr")
                    nc.tensor.transpose(tp[:], f2T[:, nt * 128:(nt + 1) * 128], ident[:])
                    nc.vector.tensor_add(out=hout_sb[:, nt, :], in0=tp[:], in1=h1_sb[:, nt, :])

                nc.sync.dma_start(out=y.ap().rearrange("(t p) c -> p t c", p=128), in_=hout_sb[:])

            for _rep in range(_REPL):
                _body_once()

    nc.compile()
    return nc


_NC = None


def _get_nc():
    global _NC
    if _NC is None:
        _NC = _build(bacc.Bacc("TRN2", target_bir_lowering=False, debug=False, num_devices=NCORES))
    return _NC


def make_in_maps(inputs):
    e = np.ascontiguousarray(np.asarray(inputs["e"], dtype=np.float32))
    u = np.asarray(inputs["u"], dtype=np.float32)
    x = np.asarray(inputs["x"], dtype=np.float32)
    e_resh = np.ascontiguousarray(e.reshape(JT, 128).T)  # [p, jt] = e[jt*128+p]
    fp8_np = mybir.dt.np(FP8)

    wflat = np.concatenate([
        np.asarray(inputs[name], dtype=np.float32).ravel() for name, _ in WEIGHT_NAMES
    ])

    in_maps = []
    for m in range(NCORES):
        pk = np.empty(PACK_TOTAL, np.float32)
        u8 = (u[m * ROWS:(m + 1) * ROWS] * U8_SCALE).astype(fp8_np)   # [ROWS, N]
        pk[_PACK_OFF["u8_s"]:_PACK_OFF["u8_s"] + U8_SLOTS].view(fp8_np)[:] = u8.ravel()
        pk[_PACK_OFF["uT8_s"]:_PACK_OFF["uT8_s"] + U8_SLOTS].view(fp8_np)[:] = \
            np.ascontiguousarray(u8.T).ravel()
        pk[_PACK_OFF["xT_s"]:_PACK_OFF["xT_s"] + NFEAT * ROWS] = \
            x[m * ROWS:(m + 1) * ROWS].T.ravel()
        pk[_PACK_OFF["e_js"]:_PACK_OFF["e_js"] + 128 * JTC] = \
            np.ascontiguousarray(e_resh[:, m * JTC:(m + 1) * JTC]).ravel()
        pk[_PACK_OFF["fe_w1"]:] = wflat
        in_maps.append({"pk": pk})
    return in_maps


def kernel(**inputs):
    nc = _get_nc()
    in_maps = make_in_maps(inputs)

    import os
    trace = bool(int(os.environ.get("KERNEL_TRACE", "0")))
    res = run_bass_kernel_spmd(nc, in_maps, core_ids=list(range(NCORES)), trace=trace)
    if trace and res.exec_time_ns is not None:
        print(f"HW exec time: {res.exec_time_ns} ns")
        if res.instructions_and_trace is not None:
            print("trace:", res.instructions_and_trace[1])
    out = np.concatenate([r["y"] for r in res.results], axis=0)
    return out.astype(np.float32)


# revision 7
# speedup vs baseline: 1.5618x; 1.5618x over previous
"""TRN2 Bass/Tile kernel for nn_NoFoDifformer (8-core SPMD, row-sharded).

Per core m (rows R_m = [m*1024, (m+1)*1024)):
  - All inputs arrive in ONE packed buffer (pk) per core: u is pre-cast
    host-side to fp8 e4m3 (x32 prescale) in BOTH layouts (u8 row-major for
    pass 1, uT8 column-major for pass 2), x is pre-transposed. The single
    buffer keeps the per-dispatch operand count at 2 (pk + y).
  - pass 1 (utx^T += (h/32)^T @ u8_panel): fp8 panels stream on the Pool
    SWDGE queue (0.5 MB each); matmuls at bf16 rate; psum copied to bf16
    utxT (exact scale: the 1/32 on h cancels the x32 on u).
  - new_e (sine eigen-encoding, Cody-Waite + ACT Sin) is jt-sharded, emitted
    first (DVE-only start), AllGathered early; the pass-2 1/32 descale is
    folded into its weights.
  - utx^T is all-reduced in bf16 in KERNEL_NAR (default 4) pipelined chunks
    as their panels complete; k^T v and new_e ride small collectives
    staggered into the Pool FIFO between panel loads.
  - pass 2 (h_fur^T += g16^T @ uT8): uT8 is DMA'd once (8.4 MB) into SBUF;
    per chunk as the AllReduce lands: g16 = transpose(utxT tile)*(new_e/32),
    bf16 x fp8 matmuls. No on-chip u transposes at all.
  - fp8 on the u path contributes ~5e-5 of the output scale (gate 2e-2);
    the attention path stays fp32.
"""

import numpy as np

import concourse.bacc as bacc
import concourse.mybir as mybir
import concourse.tile as tile
from concourse.bass_utils import run_bass_kernel_spmd
from concourse.masks import make_identity

F32 = mybir.dt.float32
BF16 = mybir.dt.bfloat16
FP8 = mybir.dt.float8e4
AF = mybir.ActivationFunctionType
ALU = mybir.AluOpType

NCORES = 8
N = 8192
NFEAT = 512
HID = 256
C = 128
DIM = 32
KPOW = 10
ROWS = N // NCORES      # 1024 rows per core
NT = ROWS // 128        # 8 row tiles
JT = N // 128           # 64 column tiles
PW = 512                # pass-1 panel width
JP = N // PW            # 16 panels
JTC = JT // NCORES      # 8 jt per core for new_e sharding
LN_EPS = 1e-5

U8_SCALE = 32.0         # fp8 prescale on u; cancelled via h/32 and new_e/32

TWO_PI = 6.283185307179586
INV_2PI = 1.0 / TWO_PI
CW_C1 = 6.28125
CW_C2 = float(np.float32(TWO_PI - CW_C1))
CW_C3 = TWO_PI - CW_C1 - CW_C2
MAGIC = 12582912.0      # 1.5 * 2**23, round-to-nearest trick
HALF_PI = float(np.float32(np.pi / 2))
PI_F = float(np.float32(np.pi))

WEIGHT_NAMES = [
    ("fe_w1", [NFEAT, HID]), ("fe_b1", [HID]),
    ("fe_w2", [HID, C]), ("fe_b2", [C]),
    ("eig_w", [KPOW, DIM + 1]), ("eig_b", [KPOW]), ("alpha_w", [KPOW]),
    ("mha_g", [C]), ("mha_b", [C]), ("ffn_g", [C]), ("ffn_b", [C]),
    ("wq", [C, C]), ("bq", [C]), ("wk", [C, C]), ("bk", [C]),
    ("wv", [C, C]), ("bv", [C]), ("wo", [C, C]), ("bo", [C]),
    ("f1_w", [C, C]), ("f1_b", [C]), ("f2_w", [C, C]), ("f2_b", [C]),
]

# Packed fp32-typed input layout (offsets in fp32 slots). The two u copies
# are fp8 bytes bitcast-viewed on device; both are 4B-aligned.
U8_SLOTS = ROWS * N // 4
_PACK_FIELDS = [
    ("u8_s", U8_SLOTS),          # fp8(32*u_s), row-major [ROWS, N]
    ("uT8_s", U8_SLOTS),         # fp8(32*u_s.T), [N, ROWS]
    ("xT_s", NFEAT * ROWS),
    ("e_js", 128 * JTC),
] + [(name, int(np.prod(shape))) for name, shape in WEIGHT_NAMES]
_PACK_OFF = {}
_off = 0
for _n, _s in _PACK_FIELDS:
    _PACK_OFF[_n] = _off
    _off += _s
PACK_TOTAL = _off
_PACK_SIZE = dict(_PACK_FIELDS)


def _build(nc):
    pk = nc.dram_tensor("pk", [PACK_TOTAL], F32, kind="ExternalInput")

    def io_ap(name):
        off = _PACK_OFF[name]
        return pk.ap()[off:off + _PACK_SIZE[name]]

    y = nc.dram_tensor("y", [ROWS, C], F32, kind="ExternalOutput")

    div_const = nc.inline_tensor(
        np.tile(np.arange(1, DIM // 2 + 1, dtype=np.float32), (128, 1)), name="divc"
    )

    with tile.TileContext(nc) as tc:
        with (
            tc.tile_pool(name="persist", bufs=1) as per,
            tc.tile_pool(name="pan", bufs=3) as pan,
            tc.tile_pool(name="rot", bufs=3) as rot,
            tc.tile_pool(name="g16p", bufs=4) as g16_pool,
            tc.tile_pool(name="stats", bufs=4) as stats,
            tc.tile_pool(name="psum_sm", bufs=3, space="PSUM") as psum_sm,
            tc.tile_pool(name="psum_tr", bufs=3, space="PSUM") as psum_tr,
            tc.tile_pool(name="psum_acc", bufs=1, space="PSUM") as psum_acc,
            tc.tile_pool(name="dram", bufs=1, space="DRAM") as dram,
        ):
            import os as _os
            _REPL = int(_os.environ.get("KERNEL_REPLICATE", "1"))
            NAR = int(_os.environ.get("KERNEL_NAR", "4"))
            NOCOLL = bool(int(_os.environ.get("KERNEL_NOCOLL", "0")))
            NOPASS1 = bool(int(_os.environ.get("KERNEL_NOPASS1", "0")))
            NOPASS2 = bool(int(_os.environ.get("KERNEL_NOPASS2", "0")))
            NOEPI = bool(int(_os.environ.get("KERNEL_NOEPI", "0")))
            JPC = JP // NAR              # panels per chunk
            CW = N // NAR                # columns per chunk
            JTCW = CW // 128             # jt tiles per chunk

            def _body_once():
                rg = [list(range(NCORES))]

                # ---------------- constants / weights to SBUF ----------------
                ident = per.tile([128, 128], F32, tag="ident")
                make_identity(nc, ident[:])
                ident16 = per.tile([128, 128], BF16, tag="ident16")
                make_identity(nc, ident16[:])

                eps_sb = per.tile([128, 1], F32, tag="eps_sb")
                nc.vector.memset(eps_sb[:], LN_EPS)

                div_sb = per.tile([128, DIM // 2], F32, tag="div_sb")
                nc.scalar.dma_start(out=div_sb[:], in_=div_const.ap())

                def bcast(name, width, tag):
                    t = per.tile([128, width], F32, tag=tag)
                    nc.scalar.dma_start(out=t[:], in_=io_ap(name).partition_broadcast(128))
                    return t

                def per_part(name, tag):
                    t = per.tile([128, 1], F32, tag=tag)
                    nc.scalar.dma_start(out=t[:], in_=io_ap(name).rearrange("(p o) -> p o", o=1))
                    return t

                # ---------------- new_e (jt-sharded) first: DVE-only start ----------------
                eigw_bc = bcast("eig_w", KPOW * (DIM + 1), "eigw_bc")
                eigb_bc = bcast("eig_b", KPOW, "eigb_bc")
                alpha_bc = bcast("alpha_w", KPOW, "alpha_bc")

                w2s = per.tile([128, KPOW, DIM // 2], F32, tag="w2s")
                w2c = per.tile([128, KPOW, DIM // 2], F32, tag="w2c")
                eigw_3d = eigw_bc[:].rearrange("p (k d) -> p k d", d=DIM + 1)
                alpha_b3 = alpha_bc[:].unsqueeze(2).broadcast_to([128, KPOW, DIM // 2])
                nc.vector.tensor_tensor(out=w2s[:], in0=alpha_b3, in1=eigw_3d[:, :, 1:1 + DIM // 2], op=ALU.mult)
                nc.vector.tensor_tensor(out=w2c[:], in0=alpha_b3, in1=eigw_3d[:, :, 1 + DIM // 2:DIM + 1], op=ALU.mult)
                # fold the fp8 u descale (1/U8_SCALE) into the sine weights
                nc.vector.tensor_scalar_mul(out=w2s[:], in0=w2s[:], scalar1=1.0 / U8_SCALE)
                nc.vector.tensor_scalar_mul(out=w2c[:], in0=w2c[:], scalar1=1.0 / U8_SCALE)
                w0t = per.tile([128, KPOW], F32, tag="w0t")
                nc.vector.tensor_tensor(out=w0t[:], in0=eigw_3d[:, :, 0], in1=eigb_bc[:], op=ALU.add)
                nc.vector.tensor_tensor(out=w0t[:], in0=w0t[:], in1=alpha_bc[:], op=ALU.mult)
                w0 = per.tile([128, 1], F32, tag="w0")
                nc.vector.tensor_reduce(out=w0[:], in_=w0t[:], axis=mybir.AxisListType.X, op=ALU.add)
                nc.vector.tensor_scalar_mul(out=w0[:], in0=w0[:], scalar1=1.0 / U8_SCALE)

                e_sb = per.tile([128, JTC], F32, tag="e_sb")
                nc.scalar.dma_start(out=e_sb[:], in_=io_ap("e_js").rearrange("(p w) -> p w", p=128))
                pows = per.tile([128, JTC, KPOW], F32, tag="pows")
                nc.vector.tensor_copy(out=pows[:, :, 0], in_=e_sb[:])
                for k in range(1, KPOW):
                    nc.vector.tensor_tensor(out=pows[:, :, k], in0=pows[:, :, k - 1], in1=e_sb[:], op=ALU.mult)

                WNE = JTC * KPOW * (DIM // 2)  # 1280
                pe_t = per.tile([128, JTC, KPOW, DIM // 2], F32, tag="pe_t")
                kq_t = per.tile([128, WNE], F32, tag="kq_t")
                trig = per.tile([128, WNE], F32, tag="trig")
                ne_s = per.tile([128, JTC], F32, tag="ne_s")
                ne_c = per.tile([128, JTC], F32, tag="ne_c")

                pows_b = pows[:].unsqueeze(3).broadcast_to([128, JTC, KPOW, DIM // 2])
                div_b = div_sb[:].unsqueeze(1).unsqueeze(1).broadcast_to([128, JTC, KPOW, DIM // 2])
                nc.vector.tensor_tensor(out=pe_t[:], in0=pows_b, in1=div_b, op=ALU.mult)
                pe_f = pe_t[:].rearrange("p a b c -> p (a b c)")
                nc.vector.tensor_scalar(out=kq_t[:], in0=pe_f, scalar1=INV_2PI, scalar2=MAGIC, op0=ALU.mult, op1=ALU.add)
                nc.vector.tensor_scalar_sub(out=kq_t[:], in0=kq_t[:], scalar1=MAGIC)
                # range-reduce pe in place: pe -= k*(c1+c2+c3)
                nc.vector.cody_waite_cascade(pe_f, pe_f, kq_t[:], CW_C1, CW_C2, CW_C3)

                w2s_b = w2s[:].rearrange("p k d -> p (k d)").unsqueeze(1).broadcast_to([128, JTC, KPOW * DIM // 2])
                w2c_b = w2c[:].rearrange("p k d -> p (k d)").unsqueeze(1).broadcast_to([128, JTC, KPOW * DIM // 2])

                nc.scalar.activation(out=trig[:], in_=pe_f, func=AF.Sin)
                trig3 = trig[:].rearrange("p (a w) -> p a w", a=JTC)
                nc.vector.tensor_tensor(out=trig3, in0=trig3, in1=w2s_b, op=ALU.mult)
                nc.vector.tensor_reduce(out=ne_s[:], in_=trig3, axis=mybir.AxisListType.X, op=ALU.add)

                nc.vector.add_range_wrap(kq_t[:], pe_f, HALF_PI, PI_F, TWO_PI)
                nc.scalar.activation(out=trig[:], in_=kq_t[:], func=AF.Sin)
                nc.vector.tensor_tensor(out=trig3, in0=trig3, in1=w2c_b, op=ALU.mult)
                nc.vector.tensor_reduce(out=ne_c[:], in_=trig3, axis=mybir.AxisListType.X, op=ALU.add)

                nc.vector.tensor_tensor(out=ne_s[:], in0=ne_s[:], in1=ne_c[:], op=ALU.add)
                nc.vector.tensor_scalar_add(out=ne_s[:], in0=ne_s[:], scalar1=w0[:])

                ag_in = dram.tile([128 * JTC], F32, tag="ag_in")
                ag_out = dram.tile([N], F32, tag="ag_out", addr_space="Shared")
                new_e_sb = per.tile([128, JT], F32, tag="new_e_sb")

                def emit_ag():
                    nc.sync.dma_start(out=ag_in[:].rearrange("(p w) -> p w", p=128), in_=ne_s[:])
                    if not NOCOLL:
                        nc.gpsimd.collective_compute(
                            "AllGather", ALU.bypass, replica_groups=rg,
                            ins=[ag_in[:].opt()], outs=[ag_out[:].opt()],
                        )
                    nc.scalar.dma_start(
                        out=new_e_sb[:].rearrange("p (m w) -> p m w", w=JTC),
                        in_=ag_out[:].rearrange("(m p w) -> p m w", p=128, w=JTC),
                    )

                # ---------------- encoder: h = relu(x@w1+b1)@w2+b2 ----------------
                w1_sb = per.tile([128, NFEAT // 128, HID], F32, tag="w1_sb")
                nc.sync.dma_start(out=w1_sb[:], in_=io_ap("fe_w1").rearrange("(t p h) -> p t h", p=128, h=HID))
                w2_sb = per.tile([128, HID // 128, C], F32, tag="w2_sb")
                nc.sync.dma_start(out=w2_sb[:], in_=io_ap("fe_w2").rearrange("(t p c) -> p t c", p=128, c=C))
                b1_sb = per.tile([128, HID // 128], F32, tag="b1_sb")
                nc.sync.dma_start(out=b1_sb[:], in_=io_ap("fe_b1").rearrange("(t p) -> p t", p=128))
                b2_bc = bcast("fe_b2", C, "b2_bc")

                wq_sb = per.tile([128, C], F32, tag="wq_sb")
                nc.sync.dma_start(out=wq_sb[:], in_=io_ap("wq").rearrange("(p c) -> p c", c=C))
                wk_sb = per.tile([128, C], F32, tag="wk_sb")
                nc.sync.dma_start(out=wk_sb[:], in_=io_ap("wk").rearrange("(p c) -> p c", c=C))
                wv_sb = per.tile([128, C], F32, tag="wv_sb")
                nc.sync.dma_start(out=wv_sb[:], in_=io_ap("wv").rearrange("(p c) -> p c", c=C))
                wo_sb = per.tile([128, C], F32, tag="wo_sb")
                nc.sync.dma_start(out=wo_sb[:], in_=io_ap("wo").rearrange("(p c) -> p c", c=C))
                f1w_sb = per.tile([128, C], F32, tag="f1w_sb")
                nc.sync.dma_start(out=f1w_sb[:], in_=io_ap("f1_w").rearrange("(p c) -> p c", c=C))
                f2w_sb = per.tile([128, C], F32, tag="f2w_sb")
                nc.sync.dma_start(out=f2w_sb[:], in_=io_ap("f2_w").rearrange("(p c) -> p c", c=C))

                bq_pp = per_part("bq", "bq_pp")
                bo_pp = per_part("bo", "bo_pp")
                f1b_pp = per_part("f1_b", "f1b_pp")
                f2b_pp = per_part("f2_b", "f2b_pp")
                bk_bc = bcast("bk", C, "bk_bc")
                bv_bc = bcast("bv", C, "bv_bc")
                mhag_bc = bcast("mha_g", C, "mhag_bc")
                mhab_bc = bcast("mha_b", C, "mhab_bc")
                ffng_bc = bcast("ffn_g", C, "ffng_bc")
                ffnb_bc = bcast("ffn_b", C, "ffnb_bc")

                # xT arrives pre-transposed: [f_part, 4(ft), n]
                xT = per.tile([128, NFEAT // 128, ROWS], F32, tag="xT")
                nc.sync.dma_start(out=xT[:], in_=io_ap("xT_s").rearrange("(t p n) -> p t n", p=128, n=ROWS))

                # uT8 (pass-2 moving operand) as one 8.4 MB DMA
                uT8 = per.tile([128, JT, ROWS], FP8, tag="uT8")
                nc.sync.dma_start(
                    out=uT8[:],
                    in_=io_ap("uT8_s").bitcast(FP8).rearrange("(t p i) -> p t i", p=128, i=ROWS),
                )

                # t1^T [hid_part, 2(ht), n] = relu(w1^T x^T + b1)
                t1T = per.tile([128, HID // 128, ROWS], F32, tag="t1T")
                for ht in range(HID // 128):
                    for nch in range(ROWS // 512):
                        ps = psum_sm.tile([128, 512], F32, tag="ps_sm")
                        for ft in range(NFEAT // 128):
                            nc.tensor.matmul(
                                ps[:], lhsT=w1_sb[:, ft, ht * 128:(ht + 1) * 128],
                                rhs=xT[:, ft, nch * 512:(nch + 1) * 512],
                                start=(ft == 0), stop=(ft == NFEAT // 128 - 1),
                            )
                        nc.scalar.activation(
                            out=t1T[:, ht, nch * 512:(nch + 1) * 512], in_=ps[:],
                            func=AF.Relu, bias=b1_sb[:, ht:ht + 1],
                        )

                # h [n_part, 8(nt), C] = t1 @ w2 + b2
                h_sb = per.tile([128, NT, C], F32, tag="h_sb")
                for nt in range(NT):
                    ps = psum_sm.tile([128, C], F32, tag="ps_sm")
                    for ht in range(HID // 128):
                        nc.tensor.matmul(
                            ps[:], lhsT=t1T[:, ht, nt * 128:(nt + 1) * 128],
                            rhs=w2_sb[:, ht, :],
                            start=(ht == 0), stop=(ht == HID // 128 - 1),
                        )
                    nc.vector.tensor_add(out=h_sb[:, nt, :], in0=ps[:], in1=b2_bc[:])

                # h16 = h / U8_SCALE: cancels the x32 on the fp8 u panels
                h16_sb = per.tile([128, NT, C], BF16, tag="h16_sb")
                for nt in range(NT):
                    nc.vector.tensor_scalar_mul(out=h16_sb[:, nt, :], in0=h_sb[:, nt, :], scalar1=1.0 / U8_SCALE)

                # ---------------- LN1 + q/k/v + kTv partial ----------------
                def layer_norm(src, dst, g_bc, b_bc):
                    for nt in range(NT):
                        st = stats.tile([128, 6], F32, tag="ln_st")
                        nc.vector.bn_stats(out=st[:], in_=src[:, nt, :])
                        mv = stats.tile([128, 2], F32, tag="ln_mv")
                        nc.vector.bn_aggr(out=mv[:], in_=st[:])
                        rstd = stats.tile([128, 1], F32, tag="ln_rstd")
                        nc.scalar.activation(out=rstd[:], in_=mv[:, 1:2], func=AF.Sqrt, bias=eps_sb[:])
                        nc.vector.reciprocal(out=rstd[:], in_=rstd[:])
                        nc.vector.tensor_scalar(
                            out=dst[:, nt, :], in0=src[:, nt, :],
                            scalar1=mv[:, 0:1], scalar2=rstd[:],
                            op0=ALU.subtract, op1=ALU.mult,
                        )
                        nc.vector.tensor_tensor(out=dst[:, nt, :], in0=dst[:, nt, :], in1=g_bc[:], op=ALU.mult)
                        nc.vector.tensor_tensor(out=dst[:, nt, :], in0=dst[:, nt, :], in1=b_bc[:], op=ALU.add)

                mh_sb = per.tile([128, NT, C], F32, tag="mh_sb")
                layer_norm(h_sb, mh_sb, mhag_bc, mhab_bc)

                mhT = per.tile([128, ROWS], F32, tag="mhT")
                for nt in range(NT):
                    tp = psum_tr.tile([128, 128], F32, tag="tr")
                    nc.tensor.transpose(tp[:], mh_sb[:, nt, :], ident[:])
                    nc.vector.tensor_copy(out=mhT[:, nt * 128:(nt + 1) * 128], in_=tp[:])

                qT = per.tile([128, ROWS], F32, tag="qT")
                for nch in range(ROWS // 512):
                    ps = psum_sm.tile([128, 512], F32, tag="ps_sm")
                    nc.tensor.matmul(ps[:], lhsT=wq_sb[:], rhs=mhT[:, nch * 512:(nch + 1) * 512], start=True, stop=True)
                    nc.scalar.activation(out=qT[:, nch * 512:(nch + 1) * 512], in_=ps[:], func=AF.Identity, bias=bq_pp[:])

                k_sb = per.tile([128, NT, C], F32, tag="k_sb")
                v_sb = per.tile([128, NT, C], F32, tag="v_sb")
                for nt in range(NT):
                    ps = psum_sm.tile([128, C], F32, tag="ps_sm")
                    nc.tensor.matmul(ps[:], lhsT=mhT[:, nt * 128:(nt + 1) * 128], rhs=wk_sb[:], start=True, stop=True)
                    nc.vector.tensor_add(out=k_sb[:, nt, :], in0=ps[:], in1=bk_bc[:])
                    ps2 = psum_sm.tile([128, C], F32, tag="ps_sm")
                    nc.tensor.matmul(ps2[:], lhsT=mhT[:, nt * 128:(nt + 1) * 128], rhs=wv_sb[:], start=True, stop=True)
                    nc.vector.tensor_add(out=v_sb[:, nt, :], in0=ps2[:], in1=bv_bc[:])

                kTv_sb = per.tile([128, C], F32, tag="kTv_sb")
                pskv = psum_sm.tile([128, C], F32, tag="ps_sm")
                for nt in range(NT):
                    nc.tensor.matmul(pskv[:], lhsT=k_sb[:, nt, :], rhs=v_sb[:, nt, :], start=(nt == 0), stop=(nt == NT - 1))
                nc.vector.tensor_copy(out=kTv_sb[:], in_=pskv[:])

                # ---------------- pass 1 + chunked AllReduce ----------------
                utxT = per.tile([128, N], BF16, tag="utxT")
                u8_r = io_ap("u8_s").bitcast(FP8).rearrange("(t p j) -> p t j", p=128, j=N)

                ar_ins, ar_outs = [], []
                for c in range(NAR):
                    ari = dram.tile([128, CW], BF16, tag=f"ar_in{c}", name=f"ar_in{c}")
                    aro = dram.tile([128, CW], BF16, tag=f"ar_out{c}", name=f"ar_out{c}",
                                    addr_space="Shared")
                    ar_ins.append(ari)
                    ar_outs.append(aro)
                ktv_in = dram.tile([128, C], F32, tag="ktv_in")
                ktv_out = dram.tile([128, C], F32, tag="ktv_out", addr_space="Shared")

                def emit_chunk_ar(c):
                    # input copy on sync; trigger on gpsimd (required engine for
                    # collectives); result load-back on scalar HWDGE so the sync
                    # FIFO and Pool FIFO (panel loads) don't stall on completion.
                    nc.sync.dma_start(out=ar_ins[c][:], in_=utxT[:, c * CW:(c + 1) * CW])
                    if not NOCOLL:
                        nc.gpsimd.collective_compute(
                            "AllReduce", ALU.add, replica_groups=rg,
                            ins=[ar_ins[c][:].opt()], outs=[ar_outs[c][:].opt()],
                        )
                    nc.scalar.dma_start(out=utxT[:, c * CW:(c + 1) * CW], in_=ar_outs[c][:])

                def emit_ktv_ar():
                    nc.sync.dma_start(out=ktv_in[:], in_=kTv_sb[:])
                    if not NOCOLL:
                        nc.gpsimd.collective_compute(
                            "AllReduce", ALU.add, replica_groups=rg,
                            ins=[ktv_in[:].opt()], outs=[ktv_out[:].opt()],
                        )
                    nc.scalar.dma_start(out=kTv_sb[:], in_=ktv_out[:])

                for jp in range(JP):
                    panel = pan.tile([128, NT, PW], FP8, tag="panel")
                    nc.gpsimd.dma_start(out=panel[:], in_=u8_r[:, :, jp * PW:(jp + 1) * PW])
                    # stagger collective triggers between panel loads so the
                    # gpsimd sequencer's wait overlaps in-flight panel loads
                    if jp == 2:
                        emit_ag()
                    if jp == 3:
                        emit_ktv_ar()
                    if jp >= JPC + 2 and (jp - JPC - 2) % JPC == 0 and (jp - JPC - 2) // JPC < NAR - 1:
                        emit_chunk_ar((jp - JPC - 2) // JPC)
                    if not NOPASS1:
                        ps = psum_sm.tile([128, PW], F32, tag="ps_sm")
                        for nt in range(NT):
                            nc.tensor.matmul(
                                ps[:], lhsT=h16_sb[:, nt, :],
                                rhs=panel[:, nt, :],
                                start=(nt == 0), stop=(nt == NT - 1),
                            )
                        nc.vector.tensor_copy(out=utxT[:, jp * PW:(jp + 1) * PW], in_=ps[:])
                emit_chunk_ar(NAR - 1)

                # ---------------- pass 2: h_fur^T += g16^T @ uT8 ----------------
                hfur_ps = psum_acc.tile([128, ROWS], F32, tag="hfur")
                if NOPASS2:
                    for hf in range(ROWS // 512):
                        nc.tensor.matmul(
                            hfur_ps[:, hf * 512:(hf + 1) * 512], lhsT=h16_sb[:, 0, :],
                            rhs=uT8[:, 0, hf * 512:(hf + 1) * 512],
                            start=True, stop=True, skip_group_check=True,
                        )
                for c in range(NAR if not NOPASS2 else 0):
                    for jtl in range(JTCW):
                        jt = c * JTCW + jtl
                        tp = psum_tr.tile([128, 128], BF16, tag="tr", name="tp16")
                        nc.tensor.transpose(tp[:], utxT[:, jt * 128:(jt + 1) * 128], ident16[:])
                        g16 = g16_pool.tile([128, 128], BF16, tag="g16")
                        if jt % 2 == 0:
                            nc.vector.tensor_scalar_mul(out=g16[:], in0=tp[:], scalar1=new_e_sb[:, jt:jt + 1])
                        else:
                            nc.scalar.activation(out=g16[:], in_=tp[:], func=AF.Identity, scale=new_e_sb[:, jt:jt + 1])
                        for hf in range(ROWS // 512):
                            nc.tensor.matmul(
                                hfur_ps[:, hf * 512:(hf + 1) * 512], lhsT=g16[:],
                                rhs=uT8[:, jt, hf * 512:(hf + 1) * 512],
                                start=(jt == 0), stop=(jt == JT - 1),
                                skip_group_check=True,
                            )

                # ---------------- att^T, att2^T + h_fur^T -> s^T; h1 = h + s ----------------
                hfurT = rot.tile([128, ROWS], F32, tag="bigT", name="hfurT")
                nc.vector.tensor_copy(out=hfurT[:], in_=hfur_ps[:])

                attT = rot.tile([128, ROWS], F32, tag="bigT", name="attT")
                for nch in range(ROWS // 512):
                    ps = psum_sm.tile([128, 512], F32, tag="ps_sm")
                    nc.tensor.matmul(ps[:], lhsT=kTv_sb[:], rhs=qT[:, nch * 512:(nch + 1) * 512], start=True, stop=True)
                    nc.vector.tensor_copy(out=attT[:, nch * 512:(nch + 1) * 512], in_=ps[:])

                sT = rot.tile([128, ROWS], F32, tag="bigT", name="sT")
                for nch in range(ROWS // 512):
                    ps = psum_sm.tile([128, 512], F32, tag="ps_sm")
                    nc.tensor.matmul(ps[:], lhsT=wo_sb[:], rhs=attT[:, nch * 512:(nch + 1) * 512], start=True, stop=True)
                    nc.vector.scalar_tensor_tensor(
                        out=sT[:, nch * 512:(nch + 1) * 512], in0=ps[:], scalar=bo_pp[:],
                        in1=hfurT[:, nch * 512:(nch + 1) * 512],
                        op0=ALU.add, op1=ALU.add,
                    )

                h1_sb = per.tile([128, NT, C], F32, tag="h1_sb")
                for nt in range(NT):
                    tp = psum_tr.tile([128, 128], F32, tag="tr")
                    nc.tensor.transpose(tp[:], sT[:, nt * 128:(nt + 1) * 128], ident[:])
                    nc.vector.tensor_add(out=h1_sb[:, nt, :], in0=tp[:], in1=h_sb[:, nt, :])

                # ---------------- FFN: h_out = h1 + (gelu(LN(h1)@f1+b1))@f2+b2 ----------------
                if NOEPI:
                    nc.sync.dma_start(out=y.ap().rearrange("(t p) c -> p t c", p=128), in_=h1_sb[:])
                    return
                mh2_sb = per.tile([128, NT, C], F32, tag="mh2_sb")
                layer_norm(h1_sb, mh2_sb, ffng_bc, ffnb_bc)
                mh2T = rot.tile([128, ROWS], F32, tag="bigT", name="mh2T")
                for nt in range(NT):
                    tp = psum_tr.tile([128, 128], F32, tag="tr")
                    nc.tensor.transpose(tp[:], mh2_sb[:, nt, :], ident[:])
                    nc.vector.tensor_copy(out=mh2T[:, nt * 128:(nt + 1) * 128], in_=tp[:])

                gzT = rot.tile([128, ROWS], F32, tag="bigT", name="gzT")
                for nch in range(ROWS // 512):
                    ps = psum_sm.tile([128, 512], F32, tag="ps_sm")
                    nc.tensor.matmul(ps[:], lhsT=f1w_sb[:], rhs=mh2T[:, nch * 512:(nch + 1) * 512], start=True, stop=True)
                    nc.scalar.activation(out=gzT[:, nch * 512:(nch + 1) * 512], in_=ps[:], func=AF.Gelu, bias=f1b_pp[:])

                f2T = rot.tile([128, ROWS], F32, tag="bigT", name="f2T")
                for nch in range(ROWS // 512):
                    ps = psum_sm.tile([128, 512], F32, tag="ps_sm")
                    nc.tensor.matmul(ps[:], lhsT=f2w_sb[:], rhs=gzT[:, nch * 512:(nch + 1) * 512], start=True, stop=True)
                    nc.scalar.activation(out=f2T[:, nch * 512:(nch + 1) * 512], in_=ps[:], func=AF.Identity, bias=f2b_pp[:])

                hout_sb = per.tile([128, NT, C], F32, tag="hout_sb")
                for nt in range(NT):
                    tp = psum_tr.tile([128, 128], F32, tag="tr")
                    nc.tensor.transpose(tp[:], f2T[:, nt * 128:(nt + 1) * 128], ident[:])
                    nc.vector.tensor_add(out=hout_sb[:, nt, :], in0=tp[:], in1=h1_sb[:, nt, :])

                nc.sync.dma_start(out=y.ap().rearrange("(t p) c -> p t c", p=128), in_=hout_sb[:])

            for _rep in range(_REPL):
                _body_once()

    nc.compile()
    return nc


_NC = None


def _get_nc():
    global _NC
    if _NC is None:
        _NC = _build(bacc.Bacc("TRN2", target_bir_lowering=False, debug=False, num_devices=NCORES))
    return _NC


def make_in_maps(inputs):
    e = np.ascontiguousarray(np.asarray(inputs["e"], dtype=np.float32))
    u = np.asarray(inputs["u"], dtype=np.float32)
    x = np.asarray(inputs["x"], dtype=np.float32)
    e_resh = np.ascontiguousarray(e.reshape(JT, 128).T)  # [p, jt] = e[jt*128+p]
    fp8_np = mybir.dt.np(FP8)

    wflat = np.concatenate([
        np.asarray(inputs[name], dtype=np.float32).ravel() for name, _ in WEIGHT_NAMES
    ])

    in_maps = []
    for m in range(NCORES):
        pk = np.empty(PACK_TOTAL, np.float32)
        u8 = (u[m * ROWS:(m + 1) * ROWS] * U8_SCALE).astype(fp8_np)   # [ROWS, N]
        pk[_PACK_OFF["u8_s"]:_PACK_OFF["u8_s"] + U8_SLOTS].view(fp8_np)[:] = u8.ravel()
        pk[_PACK_OFF["uT8_s"]:_PACK_OFF["uT8_s"] + U8_SLOTS].view(fp8_np)[:] = \
            np.ascontiguousarray(u8.T).ravel()
        pk[_PACK_OFF["xT_s"]:_PACK_OFF["xT_s"] + NFEAT * ROWS] = \
            x[m * ROWS:(m + 1) * ROWS].T.ravel()
        pk[_PACK_OFF["e_js"]:_PACK_OFF["e_js"] + 128 * JTC] = \
            np.ascontiguousarray(e_resh[:, m * JTC:(m + 1) * JTC]).ravel()
        pk[_PACK_OFF["fe_w1"]:] = wflat
        in_maps.append({"pk": pk})
    return in_maps


def kernel(**inputs):
    nc = _get_nc()
    in_maps = make_in_maps(inputs)

    import os
    trace = bool(int(os.environ.get("KERNEL_TRACE", "0")))
    res = run_bass_kernel_spmd(nc, in_maps, core_ids=list(range(NCORES)), trace=trace)
    if trace and res.exec_time_ns is not None:
        print(f"HW exec time: {res.exec_time_ns} ns")
        if res.instructions_and_trace is not None:
            print("trace:", res.instructions_and_trace[1])
    out = np.concatenate([r["y"] for r in res.results], axis=0)
    return out.astype(np.float32)
